# revision 25
# baseline (speedup 1.0000x reference)
"""Cluster-local attention kernel for Trainium2 (8 NeuronCores, SPMD).

Strategy
--------
Host side (numpy, cheap): replicate the reference's static window layout
(argsort by cluster label, bincount, big-cluster splitting), pack the ragged
windows into 128-slot tiles (best-fit decreasing), and split tiles evenly
across the 8 cores.  Attention is strictly window-local, so each tile is an
independent block-diagonal attention problem described by per-slot arrays:
source row in x (gather index), window id within the tile (mask), and
validity (for host-side compaction of the output).

Device side (Bass/Tile, uniform SPMD program, per-core data):
  per 4-tile group (512 token slots):
    - indirect-DMA gather of x rows -> X [128,384] f32 (kept for residual)
    - cast to bf16, PE-transpose -> xT [128, 3, 512] channel-major
    - QKV projection (bf16 matmuls, fp32 PSUM):
        q,k channel-major [128ch, 8 chunks, 512tok] (heads padded 96->128,
        1/sqrt(D) folded into wq host-side); v token-major [128tok, 384]
    - per tile: scoresT = k_h q_h^T (transposed scores, j on partitions),
      exp on ScalarE (no max subtraction needed -- scores are O(1)),
      block-diagonal mask multiply (window-id equality; the id row is
      materialized by a PE transpose of the id column), denominator via an
      all-ones [128,96] matmul (materializes the den row broadcast across
      96 PSUM partitions for free), reciprocal on DVE, out_hT = v_h^T
      probsT with the 1/den multiply fused into the PSUM->SBUF copy,
      out-projection, residual add, contiguous store to a staging buffer.
Host compacts the staging buffers (drops pad slots) and scatters rows to
their cluster-sorted positions (the reference returns cluster-sorted order).

The Pool engine is used ONLY to issue indirect-gather DMAs -- this image
ships no GpSimd HIPI ucode, so no Pool compute instructions are used.
"""

import os
import sys

sys.path.insert(0, "/opt/trn_rl_repo")

import numpy as np
import ml_dtypes

import concourse.bass as bass
import concourse.tile as tile
from concourse import bacc
from concourse import mybir
from concourse.bass import IndirectOffsetOnAxis
from concourse.bass_utils import run_bass_kernel_spmd

# problem constants (hardcoded per harness contract)
B, L, C, H, NCLUST, CS = 1, 32768, 384, 4, 512, 64
D = C // H  # 96
P = 128
NCORES = 8

f32 = mybir.dt.float32
bf16 = mybir.dt.bfloat16
i32 = mybir.dt.int32
f8 = mybir.dt.float8e4
F8_NP = ml_dtypes.float8_e4m3
W_SCALE = 64.0

_last_results = None  # stash of BassKernelResults for test.py introspection


# ----------------------------------------------------------------------------
# host-side window layout (replicates reference._windows)
# ----------------------------------------------------------------------------

def _window_layout(labels):
    """Return (index, starts, sizes) of the ragged windows over sorted order."""
    index = np.argsort(labels, kind="stable")
    sizes = np.bincount(labels).tolist()
    new = []
    for s in sizes:
        if s >= CS * 2:
            n = s // CS
            q, r = divmod(s, n)
            new.extend([q + 1 if i < r else q for i in range(n)])
        elif s > 0:
            new.append(s)
    sizes = np.asarray(new, np.int64)
    starts = np.concatenate([[0], np.cumsum(sizes)[:-1]])
    return index, starts, sizes


def _pack_bins(sizes, cap=P):
    """Best-fit decreasing packing of windows into bins of `cap` slots."""
    order = np.argsort(-sizes, kind="stable")
    rem = []
    bins = []
    for w in order:
        s = int(sizes[w])
        best = -1
        best_rem = cap + 1
        for bi, r in enumerate(rem):
            if s <= r < best_rem:
                best, best_rem = bi, r
        if best < 0:
            bins.append([int(w)])
            rem.append(cap - s)
        else:
            bins[best].append(int(w))
            rem[best] -= s
    return bins


def _build_core_data(labels):
    """Build per-core slot arrays (see module docstring)."""
    index, starts, sizes = _window_layout(labels)
    bins = _pack_bins(sizes)
    core_bins = [bins[c::NCORES] for c in range(NCORES)]
    T = max(len(cb) for cb in core_bins)

    per_core = []
    for c in range(NCORES):
        srci = np.zeros((T, P), np.int32)
        wid = np.full((T, P), -1.0, np.float32)
        valid = np.zeros((T, P), bool)
        gpos = []
        for t, b in enumerate(core_bins[c]):
            off = 0
            for k, w in enumerate(b):
                s = int(sizes[w])
                sl = slice(off, off + s)
                srci[t, sl] = index[starts[w]: starts[w] + s]
                wid[t, sl] = float(k)
                valid[t, sl] = True
                gpos.extend(range(int(starts[w]), int(starts[w]) + s))
                off += s
        eqm = (wid[:, :, None] == wid[:, None, :]).astype(ml_dtypes.bfloat16)
        per_core.append(dict(
            srci=np.ascontiguousarray(srci.T),          # [P, T]
            eqm=np.ascontiguousarray(eqm.reshape(T * P, P)),
            gpos=np.asarray(gpos, np.int64),
            valid=valid.reshape(-1),
        ))
    return T, per_core


# ----------------------------------------------------------------------------
# weight reorganization (host side)
# ----------------------------------------------------------------------------

def _prep_weights(w_qkv, w_out):
    """Reorganize weights into the SBUF layouts the kernel consumes.

    q/k and v weights are stored fp8(e4m3) scaled by W_SCALE (the raw weights
    are ~N(0, 0.02) -- subnormal in fp8); the scale is divided back out on the
    PSUM->SBUF evacuation (with q additionally carrying 1/sqrt(D))."""
    wq = w_qkv[:, :C]
    wk = w_qkv[:, C:2 * C]
    wv = w_qkv[:, 2 * C:]

    qk = np.zeros((C, 8, P), np.float32)
    for h in range(H):
        qk[:, h, :D] = wq[:, h * D:(h + 1) * D]
        qk[:, 4 + h, :D] = wk[:, h * D:(h + 1) * D]
    qk = qk.reshape(C, 8 * P) * W_SCALE               # [384, 1024]
    wqk = qk.reshape(3, P, 8 * P).transpose(1, 0, 2)  # [128, 3, 1024]

    wv_r = wv.reshape(3, P, C).transpose(1, 0, 2) * W_SCALE  # [128, 3, 384]
    wo_r = w_out.reshape(H, D, C).transpose(1, 0, 2) * W_SCALE  # [96,4,384]

    bf = ml_dtypes.bfloat16
    return (np.ascontiguousarray(wqk).astype(F8_NP),
            np.ascontiguousarray(wv_r).astype(F8_NP),
            np.ascontiguousarray(wo_r).astype(bf))


# ----------------------------------------------------------------------------
# Bass program
# ----------------------------------------------------------------------------

def build_program(T):
    """Build the uniform SPMD Bass program for T tiles."""
    G = (T + 3) // 4
    nc = bacc.Bacc("TRN2", target_bir_lowering=False)

    xg = nc.dram_tensor("xg", [T * P, C], f32, kind="ExternalInput")
    xgT = nc.dram_tensor("xgT", [(T + 3) // 4, P, 12 * P], f8, kind="ExternalInput")
    wqk = nc.dram_tensor("wqk", [P, 3, 8 * P], f8, kind="ExternalInput")
    wv = nc.dram_tensor("wv", [P, 3, C], f8, kind="ExternalInput")
    wo = nc.dram_tensor("wo", [D, H, C], bf16, kind="ExternalInput")
    eqm = nc.dram_tensor("eqm", [T * P, P], bf16, kind="ExternalInput")
    ones96 = nc.dram_tensor("ones96", [P, D], bf16, kind="ExternalInput")
    y_out = nc.dram_tensor("y", [T * P, C], f32, kind="ExternalOutput")

    ExpF = mybir.ActivationFunctionType.Exp

    with tile.TileContext(nc) as tc:
        with (
            tc.tile_pool(name="const", bufs=1) as cpool,
            tc.tile_pool(name="sb", bufs=2) as pool,
            tc.tile_pool(name="ps", bufs=1, space="PSUM") as psum,
        ):
            # ---- constants ----
            wqk_sb = cpool.tile([P, 3, 8 * P], f8)
            nc.sync.dma_start(wqk_sb[:], wqk[:])
            wv_sb = cpool.tile([P, 3, C], f8)
            nc.sync.dma_start(wv_sb[:], wv[:])
            wo_sb = cpool.tile([D, H, C], bf16)
            nc.sync.dma_start(wo_sb[:], wo[:])
            ones96_sb = cpool.tile([P, D], bf16)
            nc.sync.dma_start(ones96_sb[:], ones96[:])

            # ---- PE warmups ----
            # TRN2 Matmult carries at most one sync-wait command, so make the
            # PE observe each constant's DMA semaphore via a single-input
            # dummy op before any real matmul consumes it.
            w1 = psum.tile([P, P], f32, tag="small", space="PSUM")
            nc.tensor.matmul(w1[:], lhsT=wqk_sb[:, 0, 0:P], rhs=wqk_sb[:, 0, 0:P],
                             start=True, stop=True)
            w2 = psum.tile([P, P], f32, tag="small", space="PSUM")
            nc.tensor.matmul(w2[:], lhsT=wv_sb[:, 0, 0:P], rhs=wv_sb[:, 0, 0:P],
                             start=True, stop=True)
            w3 = psum.tile([P, P], f32, tag="small", space="PSUM")
            nc.tensor.matmul(w3[:], lhsT=wo_sb[:, 0, 0:P], rhs=wo_sb[:, 0, 0:P],
                             start=True, stop=True)
            w4 = psum.tile([D, D], f32, tag="small", space="PSUM")
            nc.tensor.matmul(w4[:], lhsT=ones96_sb[:], rhs=ones96_sb[:],
                             start=True, stop=True)
            for g in range(G):
                W = min(4, T - g * 4)  # tiles in this group
                # ---- load pre-packed channel-major x group [128, 3, W*128] ----
                xTg = pool.tile([P, 3, 4 * P], f8, tag="xTg", bufs=3)
                nc.sync.dma_start(
                    xTg[:, :, 0:W * P],
                    xgT[g].rearrange("p (c t) -> p c t", c=3)[:, :, 0:W * P])
                Xfs = []
                for tt in range(W):
                    t = g * 4 + tt
                    Xf = pool.tile([P, C], f32, tag="Xf", bufs=8)
                    nc.sync.dma_start(Xf[:], xg[t * P:(t + 1) * P, :])
                    Xfs.append(Xf)

                # ---- q/k projection: channel-major [128, 8, 512] ----
                qkT = pool.tile([P, 8, 4 * P], bf16, tag="qkT", bufs=3)
                for ch in range(8):
                    pqk = psum.tile([P, 4 * P], f32, tag="qk", space="PSUM", bufs=2)
                    nc.tensor.matmul(
                        pqk[:, 0:W * P],
                        lhsT=wqk_sb[:, 0:2, ch * P:(ch + 1) * P],
                        rhs=xTg[:, 0:2, 0:W * P],
                        start=True,
                        stop=False,
                        perf_mode=mybir.MatmulPerfMode.DoubleRow,
                    )
                    nc.tensor.matmul(
                        pqk[:, 0:W * P],
                        lhsT=wqk_sb[:, 2, ch * P:(ch + 1) * P],
                        rhs=xTg[:, 2, 0:W * P],
                        start=False,
                        stop=True,
                    )
                    sc = 1.0 / (W_SCALE * np.sqrt(D)) if ch < 4 else 1.0 / W_SCALE
                    if ch % 2 == 0:
                        nc.scalar.activation(
                            qkT[:, ch, 0:W * P], pqk[:, 0:W * P],
                            mybir.ActivationFunctionType.Copy, bias=0.0,
                            scale=float(sc))
                    else:
                        nc.vector.tensor_scalar_mul(
                            qkT[:, ch, 0:W * P], pqk[:, 0:W * P], float(sc))

                # ---- per-tile: v projection + attention + out ----
                for tt in range(W):
                    t = g * 4 + tt
                    tok = slice(tt * P, (tt + 1) * P)

                    # v token-major [128, 384]
                    pv = psum.tile([P, C], f32, tag="v", space="PSUM")
                    nc.tensor.matmul(
                        pv[:],
                        lhsT=xTg[:, 0:2, tok],
                        rhs=wv_sb[:, 0:2, :],
                        start=True,
                        stop=False,
                        perf_mode=mybir.MatmulPerfMode.DoubleRow,
                    )
                    nc.tensor.matmul(
                        pv[:],
                        lhsT=xTg[:, 2, tok],
                        rhs=wv_sb[:, 2, :],
                        start=False,
                        stop=True,
                    )
                    v_sb = pool.tile([P, C], bf16, tag="v_sb", bufs=4)
                    if tt % 2 == 0:
                        nc.vector.tensor_scalar_mul(v_sb[:], pv[:], 1.0 / W_SCALE)
                    else:
                        nc.scalar.activation(
                            v_sb[:], pv[:],
                            mybir.ActivationFunctionType.Copy, bias=0.0,
                            scale=float(1.0 / W_SCALE))

                    eq = pool.tile([P, P], bf16, tag="eq", bufs=4)
                    nc.sync.dma_start(eq[:], eqm[t * P:(t + 1) * P, :])

                    # scoresT[j, i] per head, laid out [128, (h i)]
                    ps = psum.tile([P, H * P], f32, tag="score", space="PSUM", bufs=2)
                    for h in range(H):
                        nc.tensor.matmul(
                            ps[:, h * P:(h + 1) * P],
                            lhsT=qkT[:, 4 + h, tok],
                            rhs=qkT[:, h, tok],
                            start=True,
                            stop=True,
                        )
                    probs = pool.tile([P, H * P], bf16, tag="probs", bufs=4)
                    nc.scalar.activation(probs[:], ps[:], ExpF)
                    nc.vector.tensor_tensor(
                        probs[:].rearrange("p (h j) -> p h j", h=H),
                        probs[:].rearrange("p (h j) -> p h j", h=H),
                        eq[:, None, :].to_broadcast([P, H, P]),
                        op=mybir.AluOpType.mult,
                    )

                    # denominator, pre-broadcast across 96 partitions:
                    # denB = ones[128,96]^T @ probsT -> [96, (h i)]
                    pden = psum.tile([D, H * P], f32, tag="small", space="PSUM")
                    nc.tensor.matmul(pden[:], lhsT=ones96_sb[:], rhs=probs[:],
                                     start=True, stop=True)
                    rden = pool.tile([D, H * P], f32, tag="rden", bufs=3)
                    nc.vector.reciprocal_approx_fast(out=rden[:], in_=pden[:])

                    # out_hT = v_h^T @ probsT -> [96, (h i)]; normalize on evac
                    po = psum.tile([P, H * P], f32, tag="oy", space="PSUM", bufs=2)
                    for h in range(H):
                        nc.tensor.matmul(
                            po[0:D, h * P:(h + 1) * P],
                            lhsT=v_sb[:, h * D:(h + 1) * D],
                            rhs=probs[:, h * P:(h + 1) * P],
                            start=True,
                            stop=True,
                        )
                    hT = pool.tile([D, H, P], bf16, tag="hT", bufs=3)
                    nc.vector.tensor_tensor(
                        hT[:].rearrange("p h j -> p (h j)"),
                        po[0:D, :],
                        rden[:],
                        op=mybir.AluOpType.mult,
                    )

                    # out projection + residual
                    py = psum.tile([P, C], f32, tag="oy", space="PSUM", bufs=2)
                    for h in range(H):
                        nc.tensor.matmul(
                            py[:],
                            lhsT=hT[:, h, :],
                            rhs=wo_sb[:, h, :],
                            start=(h == 0),
                            stop=(h == 3),
                        )
                    y = pool.tile([P, C], f32, tag="y", bufs=4)
                    nc.vector.scalar_tensor_tensor(
                        out=y[:], in0=py[:], scalar=1.0 / W_SCALE,
                        in1=Xfs[tt][:], op0=mybir.AluOpType.mult,
                        op1=mybir.AluOpType.add)
                    nc.sync.dma_start(y_out[t * P:(t + 1) * P, :], y[:])

    nc.compile()
    return nc


# ----------------------------------------------------------------------------
# public entry point
# ----------------------------------------------------------------------------

def kernel(**inputs):
    global _last_results
    x = np.asarray(inputs["x"], np.float32)
    labels = np.asarray(inputs["cluster_label"]).reshape(-1).astype(np.int64)
    w_qkv = np.asarray(inputs["w_qkv"], np.float32)
    b_qkv = np.asarray(inputs["b_qkv"], np.float32)
    w_out = np.asarray(inputs["w_out"], np.float32)
    b_out = np.asarray(inputs["b_out"], np.float32)

    if np.any(b_qkv):
        raise NotImplementedError("nonzero b_qkv not supported")

    x2d = np.ascontiguousarray(x.reshape(L, C))
    T, per_core = _build_core_data(labels)
    wqk_h, wv_h, wo_h = _prep_weights(w_qkv, w_out)
    G = (T + 3) // 4
    for pc in per_core:
        slot_src = pc["srci"].T.reshape(-1)            # [T*P] slot-order token ids
        xs_rows = x2d[slot_src]                        # [T*P, C] f32
        pc["xg"] = np.ascontiguousarray(xs_rows)
        xs_pad = np.zeros((G * 4 * P, C), np.float32)
        xs_pad[:T * P] = xs_rows
        xt = xs_pad.reshape(G, 4 * P, C).transpose(0, 2, 1)    # [G, C, 512]
        xt = xt.reshape(G, 3, P, 4 * P).transpose(0, 2, 1, 3)  # [G, 128, 3, 512]
        pc["xgT"] = np.ascontiguousarray(xt.reshape(G, P, 12 * P)).astype(F8_NP)

    nc = build_program(T)

    bf = ml_dtypes.bfloat16  # noqa: F841
    ones96_h = np.ones((P, D), np.float32).astype(bf)

    in_maps = []
    for c in range(NCORES):
        in_maps.append(dict(
            xg=per_core[c]["xg"],
            xgT=per_core[c]["xgT"],
            wqk=wqk_h,
            wv=wv_h,
            wo=wo_h,
            eqm=per_core[c]["eqm"],
            ones96=ones96_h,
        ))

    res = None
    last_err = None
    for attempt in range(3):
        try:
            res = run_bass_kernel_spmd(nc, in_maps, core_ids=list(range(NCORES)))
            break
        except Exception as e:  # transient NRT_EXEC_UNIT_UNRECOVERABLE etc.
            last_err = e
            import time as _time
            _time.sleep(2.0)
    if res is None:
        raise last_err
    _last_results = res

    out_sorted = np.empty((L, C), np.float32)
    for c in range(NCORES):
        stage = res.results[c]["y"]
        rows = stage[per_core[c]["valid"]]
        out_sorted[per_core[c]["gpos"]] = rows
    if np.any(b_out):
        out_sorted += b_out[None, :]
    return out_sorted.reshape(B, L, C)


# revision 26
# speedup vs baseline: 1.1001x; 1.1001x over previous
"""Cluster-local attention kernel for Trainium2 (8 NeuronCores, SPMD).

Strategy
--------
Host side (numpy, cheap): replicate the reference's static window layout
(argsort by cluster label, bincount, big-cluster splitting), pack the ragged
windows into 128-slot tiles (best-fit decreasing), and split tiles evenly
across the 8 cores.  Attention is strictly window-local, so each tile is an
independent block-diagonal attention problem described by per-slot arrays:
source row in x (gather index), window id within the tile (mask), and
validity (for host-side compaction of the output).

Device side (Bass/Tile, uniform SPMD program, per-core data):
  per 4-tile group (512 token slots):
    - indirect-DMA gather of x rows -> X [128,384] f32 (kept for residual)
    - cast to bf16, PE-transpose -> xT [128, 3, 512] channel-major
    - QKV projection (bf16 matmuls, fp32 PSUM):
        q,k channel-major [128ch, 8 chunks, 512tok] (heads padded 96->128,
        1/sqrt(D) folded into wq host-side); v token-major [128tok, 384]
    - per tile: scoresT = k_h q_h^T (transposed scores, j on partitions),
      exp on ScalarE (no max subtraction needed -- scores are O(1)),
      block-diagonal mask multiply (window-id equality; the id row is
      materialized by a PE transpose of the id column), denominator via an
      all-ones [128,96] matmul (materializes the den row broadcast across
      96 PSUM partitions for free), reciprocal on DVE, out_hT = v_h^T
      probsT with the 1/den multiply fused into the PSUM->SBUF copy,
      out-projection, residual add, contiguous store to a staging buffer.
Host compacts the staging buffers (drops pad slots) and scatters rows to
their cluster-sorted positions (the reference returns cluster-sorted order).

The Pool engine is used ONLY to issue indirect-gather DMAs -- this image
ships no GpSimd HIPI ucode, so no Pool compute instructions are used.
"""

import os
import sys

sys.path.insert(0, "/opt/trn_rl_repo")

import numpy as np
import ml_dtypes

import concourse.bass as bass
import concourse.tile as tile
from concourse import bacc
from concourse import mybir
from concourse.bass import IndirectOffsetOnAxis
from concourse.bass_utils import run_bass_kernel_spmd

# problem constants (hardcoded per harness contract)
B, L, C, H, NCLUST, CS = 1, 32768, 384, 4, 512, 64
D = C // H  # 96
P = 128
NCORES = 8

f32 = mybir.dt.float32
bf16 = mybir.dt.bfloat16
i32 = mybir.dt.int32
f8 = mybir.dt.float8e4
F8_NP = ml_dtypes.float8_e4m3
W_SCALE = 64.0

_last_results = None  # stash of BassKernelResults for test.py introspection


# ----------------------------------------------------------------------------
# host-side window layout (replicates reference._windows)
# ----------------------------------------------------------------------------

def _window_layout(labels):
    """Return (index, starts, sizes) of the ragged windows over sorted order."""
    index = np.argsort(labels, kind="stable")
    sizes = np.bincount(labels).tolist()
    new = []
    for s in sizes:
        if s >= CS * 2:
            n = s // CS
            q, r = divmod(s, n)
            new.extend([q + 1 if i < r else q for i in range(n)])
        elif s > 0:
            new.append(s)
    sizes = np.asarray(new, np.int64)
    starts = np.concatenate([[0], np.cumsum(sizes)[:-1]])
    return index, starts, sizes


def _pack_bins(sizes, cap=P):
    """Best-fit decreasing packing of windows into bins of `cap` slots."""
    order = np.argsort(-sizes, kind="stable")
    rem = []
    bins = []
    for w in order:
        s = int(sizes[w])
        best = -1
        best_rem = cap + 1
        for bi, r in enumerate(rem):
            if s <= r < best_rem:
                best, best_rem = bi, r
        if best < 0:
            bins.append([int(w)])
            rem.append(cap - s)
        else:
            bins[best].append(int(w))
            rem[best] -= s
    return bins


def _build_core_data(labels):
    """Build per-core slot arrays (see module docstring)."""
    index, starts, sizes = _window_layout(labels)
    bins = _pack_bins(sizes)
    core_bins = [bins[c::NCORES] for c in range(NCORES)]
    T = max(len(cb) for cb in core_bins)

    per_core = []
    for c in range(NCORES):
        srci = np.zeros((T, P), np.int32)
        wid = np.full((T, P), -1.0, np.float32)
        valid = np.zeros((T, P), bool)
        gpos = []
        for t, b in enumerate(core_bins[c]):
            off = 0
            for k, w in enumerate(b):
                s = int(sizes[w])
                sl = slice(off, off + s)
                srci[t, sl] = index[starts[w]: starts[w] + s]
                wid[t, sl] = float(k)
                valid[t, sl] = True
                gpos.extend(range(int(starts[w]), int(starts[w]) + s))
                off += s
        eqm = (wid[:, :, None] == wid[:, None, :]).astype(ml_dtypes.bfloat16)
        per_core.append(dict(
            srci=np.ascontiguousarray(srci.T),          # [P, T]
            eqm=np.ascontiguousarray(eqm.reshape(T * P, P)),
            gpos=np.asarray(gpos, np.int64),
            valid=valid.reshape(-1),
        ))
    return T, per_core


# ----------------------------------------------------------------------------
# weight reorganization (host side)
# ----------------------------------------------------------------------------

def _prep_weights(w_qkv, w_out):
    """Reorganize weights into the SBUF layouts the kernel consumes.

    q/k and v weights are stored fp8(e4m3) scaled by W_SCALE (the raw weights
    are ~N(0, 0.02) -- subnormal in fp8); the scale is divided back out on the
    PSUM->SBUF evacuation (with q additionally carrying 1/sqrt(D))."""
    wq = w_qkv[:, :C]
    wk = w_qkv[:, C:2 * C]
    wv = w_qkv[:, 2 * C:]

    qk = np.zeros((C, 8, P), np.float32)
    for h in range(H):
        qk[:, h, :D] = wq[:, h * D:(h + 1) * D]
        qk[:, 4 + h, :D] = wk[:, h * D:(h + 1) * D]
    qk = qk.reshape(C, 8 * P) * W_SCALE               # [384, 1024]
    wqk = qk.reshape(3, P, 8 * P).transpose(1, 0, 2)  # [128, 3, 1024]

    wv_r = wv.reshape(3, P, C).transpose(1, 0, 2) * W_SCALE  # [128, 3, 384]
    wo_r = w_out.reshape(H, D, C).transpose(1, 0, 2) * W_SCALE  # [96,4,384]

    bf = ml_dtypes.bfloat16
    return (np.ascontiguousarray(wqk).astype(F8_NP),
            np.ascontiguousarray(wv_r).astype(F8_NP),
            np.ascontiguousarray(wo_r).astype(bf))


# ----------------------------------------------------------------------------
# Bass program
# ----------------------------------------------------------------------------

def build_program(T):
    """Build the uniform SPMD Bass program for T tiles."""
    G = (T + 3) // 4
    nc = bacc.Bacc("TRN2", target_bir_lowering=False)

    xg = nc.dram_tensor("xg", [T * P, C], f32, kind="ExternalInput")
    xgT = nc.dram_tensor("xgT", [(T + 3) // 4, P, 12 * P], f8, kind="ExternalInput")
    wqk = nc.dram_tensor("wqk", [P, 3, 8 * P], f8, kind="ExternalInput")
    wv = nc.dram_tensor("wv", [P, 3, C], f8, kind="ExternalInput")
    wo = nc.dram_tensor("wo", [D, H, C], bf16, kind="ExternalInput")
    eqm = nc.dram_tensor("eqm", [T * P, P], bf16, kind="ExternalInput")
    ones96 = nc.dram_tensor("ones96", [P, D], bf16, kind="ExternalInput")
    y_out = nc.dram_tensor("y", [T * P, C], f32, kind="ExternalOutput")

    ExpF = mybir.ActivationFunctionType.Exp

    with tile.TileContext(nc) as tc:
        with (
            tc.tile_pool(name="const", bufs=1) as cpool,
            tc.tile_pool(name="sb", bufs=2) as pool,
            tc.tile_pool(name="ps", bufs=1, space="PSUM") as psum,
        ):
            # ---- constants ----
            wqk_sb = cpool.tile([P, 3, 8 * P], f8)
            nc.sync.dma_start(wqk_sb[:], wqk[:])
            wv_sb = cpool.tile([P, 3, C], f8)
            nc.sync.dma_start(wv_sb[:], wv[:])
            wo_sb = cpool.tile([D, H, C], bf16)
            nc.sync.dma_start(wo_sb[:], wo[:])
            ones96_sb = cpool.tile([P, D], bf16)
            nc.sync.dma_start(ones96_sb[:], ones96[:])

            # ---- PE warmups ----
            # TRN2 Matmult carries at most one sync-wait command, so make the
            # PE observe each constant's DMA semaphore via a single-input
            # dummy op before any real matmul consumes it.
            w1 = psum.tile([P, P], f32, tag="small", space="PSUM")
            nc.tensor.matmul(w1[:], lhsT=wqk_sb[:, 0, 0:P], rhs=wqk_sb[:, 0, 0:P],
                             start=True, stop=True)
            w2 = psum.tile([P, P], f32, tag="small", space="PSUM")
            nc.tensor.matmul(w2[:], lhsT=wv_sb[:, 0, 0:P], rhs=wv_sb[:, 0, 0:P],
                             start=True, stop=True)
            w3 = psum.tile([P, P], f32, tag="small", space="PSUM")
            nc.tensor.matmul(w3[:], lhsT=wo_sb[:, 0, 0:P], rhs=wo_sb[:, 0, 0:P],
                             start=True, stop=True)
            w4 = psum.tile([D, D], f32, tag="small", space="PSUM")
            nc.tensor.matmul(w4[:], lhsT=ones96_sb[:], rhs=ones96_sb[:],
                             start=True, stop=True)
            for g in range(G):
                W = min(4, T - g * 4)  # tiles in this group
                # ---- load pre-packed channel-major x group [128, 3, W*128] ----
                xTg = pool.tile([P, 3, 4 * P], f8, tag="xTg", bufs=3)
                nc.sync.dma_start(
                    xTg[:, :, 0:W * P],
                    xgT[g].rearrange("p (c t) -> p c t", c=3)[:, :, 0:W * P])
                Xfs = []
                for tt in range(W):
                    t = g * 4 + tt
                    Xf = pool.tile([P, C], f32, tag="Xf", bufs=8)
                    nc.sync.dma_start(Xf[:], xg[t * P:(t + 1) * P, :])
                    Xfs.append(Xf)

                # ---- q/k projection: channel-major [128, 8, 512] ----
                qkT = pool.tile([P, 8, 4 * P], bf16, tag="qkT", bufs=3)
                for ch in range(8):
                    pqk = psum.tile([P, 4 * P], f32, tag="qk", space="PSUM", bufs=2)
                    nc.tensor.matmul(
                        pqk[:, 0:W * P],
                        lhsT=wqk_sb[:, 0:2, ch * P:(ch + 1) * P],
                        rhs=xTg[:, 0:2, 0:W * P],
                        start=True,
                        stop=False,
                        perf_mode=mybir.MatmulPerfMode.DoubleRow,
                    )
                    nc.tensor.matmul(
                        pqk[:, 0:W * P],
                        lhsT=wqk_sb[:, 2, ch * P:(ch + 1) * P],
                        rhs=xTg[:, 2, 0:W * P],
                        start=False,
                        stop=True,
                    )
                    sc = 1.0 / (W_SCALE * np.sqrt(D)) if ch < 4 else 1.0 / W_SCALE
                    nc.scalar.activation(
                        qkT[:, ch, 0:W * P], pqk[:, 0:W * P],
                        mybir.ActivationFunctionType.Copy, bias=0.0, scale=float(sc))

                # ---- per-tile: v projection + attention + out ----
                for tt in range(W):
                    t = g * 4 + tt
                    tok = slice(tt * P, (tt + 1) * P)

                    # v token-major [128, 384]
                    pv = psum.tile([P, C], f32, tag="v", space="PSUM")
                    nc.tensor.matmul(
                        pv[:],
                        lhsT=xTg[:, 0:2, tok],
                        rhs=wv_sb[:, 0:2, :],
                        start=True,
                        stop=False,
                        perf_mode=mybir.MatmulPerfMode.DoubleRow,
                    )
                    nc.tensor.matmul(
                        pv[:],
                        lhsT=xTg[:, 2, tok],
                        rhs=wv_sb[:, 2, :],
                        start=False,
                        stop=True,
                    )
                    v_sb = pool.tile([P, C], bf16, tag="v_sb", bufs=4)
                    if tt % 2 == 0:
                        nc.vector.tensor_scalar_mul(v_sb[:], pv[:], 1.0 / W_SCALE)
                    else:
                        nc.scalar.activation(
                            v_sb[:], pv[:],
                            mybir.ActivationFunctionType.Copy, bias=0.0,
                            scale=float(1.0 / W_SCALE))

                    eq = pool.tile([P, P], bf16, tag="eq", bufs=4)
                    nc.sync.dma_start(eq[:], eqm[t * P:(t + 1) * P, :])

                    # scoresT[j, i] per head, laid out [128, (h i)]
                    ps = psum.tile([P, H * P], f32, tag="score", space="PSUM", bufs=2)
                    for h in range(H):
                        nc.tensor.matmul(
                            ps[:, h * P:(h + 1) * P],
                            lhsT=qkT[:, 4 + h, tok],
                            rhs=qkT[:, h, tok],
                            start=True,
                            stop=True,
                        )
                    probs = pool.tile([P, H * P], bf16, tag="probs", bufs=4)
                    nc.scalar.activation(probs[:], ps[:], ExpF)
                    nc.vector.tensor_tensor(
                        probs[:].rearrange("p (h j) -> p h j", h=H),
                        probs[:].rearrange("p (h j) -> p h j", h=H),
                        eq[:, None, :].to_broadcast([P, H, P]),
                        op=mybir.AluOpType.mult,
                    )

                    # denominator, pre-broadcast across 96 partitions:
                    # denB = ones[128,96]^T @ probsT -> [96, (h i)]
                    pden = psum.tile([D, H * P], f32, tag="small", space="PSUM")
                    nc.tensor.matmul(pden[:], lhsT=ones96_sb[:], rhs=probs[:],
                                     start=True, stop=True)
                    rden = pool.tile([D, H * P], f32, tag="rden", bufs=3)
                    nc.vector.reciprocal_approx_fast(out=rden[:], in_=pden[:])

                    # out_hT = v_h^T @ probsT -> [96, (h i)]; normalize on evac
                    po = psum.tile([P, H * P], f32, tag="oy", space="PSUM", bufs=2)
                    for h in range(H):
                        nc.tensor.matmul(
                            po[0:D, h * P:(h + 1) * P],
                            lhsT=v_sb[:, h * D:(h + 1) * D],
                            rhs=probs[:, h * P:(h + 1) * P],
                            start=True,
                            stop=True,
                        )
                    hT = pool.tile([D, H, P], bf16, tag="hT", bufs=3)
                    nc.vector.tensor_tensor(
                        hT[:].rearrange("p h j -> p (h j)"),
                        po[0:D, :],
                        rden[:],
                        op=mybir.AluOpType.mult,
                    )

                    # out projection + residual
                    py = psum.tile([P, C], f32, tag="oy", space="PSUM", bufs=2)
                    for h in range(H):
                        nc.tensor.matmul(
                            py[:],
                            lhsT=hT[:, h, :],
                            rhs=wo_sb[:, h, :],
                            start=(h == 0),
                            stop=(h == 3),
                        )
                    y = pool.tile([P, C], f32, tag="y", bufs=4)
                    nc.vector.scalar_tensor_tensor(
                        out=y[:], in0=py[:], scalar=1.0 / W_SCALE,
                        in1=Xfs[tt][:], op0=mybir.AluOpType.mult,
                        op1=mybir.AluOpType.add)
                    nc.sync.dma_start(y_out[t * P:(t + 1) * P, :], y[:])

    nc.compile()
    return nc


# ----------------------------------------------------------------------------
# public entry point
# ----------------------------------------------------------------------------

def kernel(**inputs):
    global _last_results
    x = np.asarray(inputs["x"], np.float32)
    labels = np.asarray(inputs["cluster_label"]).reshape(-1).astype(np.int64)
    w_qkv = np.asarray(inputs["w_qkv"], np.float32)
    b_qkv = np.asarray(inputs["b_qkv"], np.float32)
    w_out = np.asarray(inputs["w_out"], np.float32)
    b_out = np.asarray(inputs["b_out"], np.float32)

    if np.any(b_qkv):
        raise NotImplementedError("nonzero b_qkv not supported")

    x2d = np.ascontiguousarray(x.reshape(L, C))
    T, per_core = _build_core_data(labels)
    wqk_h, wv_h, wo_h = _prep_weights(w_qkv, w_out)
    G = (T + 3) // 4
    for pc in per_core:
        slot_src = pc["srci"].T.reshape(-1)            # [T*P] slot-order token ids
        xs_rows = x2d[slot_src]                        # [T*P, C] f32
        pc["xg"] = np.ascontiguousarray(xs_rows)
        xs_pad = np.zeros((G * 4 * P, C), np.float32)
        xs_pad[:T * P] = xs_rows
        xt = xs_pad.reshape(G, 4 * P, C).transpose(0, 2, 1)    # [G, C, 512]
        xt = xt.reshape(G, 3, P, 4 * P).transpose(0, 2, 1, 3)  # [G, 128, 3, 512]
        pc["xgT"] = np.ascontiguousarray(xt.reshape(G, P, 12 * P)).astype(F8_NP)

    nc = build_program(T)

    bf = ml_dtypes.bfloat16  # noqa: F841
    ones96_h = np.ones((P, D), np.float32).astype(bf)

    in_maps = []
    for c in range(NCORES):
        in_maps.append(dict(
            xg=per_core[c]["xg"],
            xgT=per_core[c]["xgT"],
            wqk=wqk_h,
            wv=wv_h,
            wo=wo_h,
            eqm=per_core[c]["eqm"],
            ones96=ones96_h,
        ))

    res = None
    last_err = None
    for attempt in range(3):
        try:
            res = run_bass_kernel_spmd(nc, in_maps, core_ids=list(range(NCORES)))
            break
        except Exception as e:  # transient NRT_EXEC_UNIT_UNRECOVERABLE etc.
            last_err = e
            import time as _time
            _time.sleep(2.0)
    if res is None:
        raise last_err
    _last_results = res

    out_sorted = np.empty((L, C), np.float32)
    for c in range(NCORES):
        stage = res.results[c]["y"]
        rows = stage[per_core[c]["valid"]]
        out_sorted[per_core[c]["gpos"]] = rows
    if np.any(b_out):
        out_sorted += b_out[None, :]
    return out_sorted.reshape(B, L, C)
